# revision 2
# baseline (speedup 1.0000x reference)
"""Trainium2 Bass kernel for nn_ExpertModule (top-1 MoE, E=4, D=1024, H=2048).

Token-parallel SPMD over 8 NeuronCores: each core owns 4096 tokens and a
replicated copy of the (small) weights. Per core:

  Phase A (gate, exact fp32 matmul):
    logitsT = Wg.T @ x.T (+bg) on the PE; PE-transpose to [tok, e]; top-1
    one-hot / max-prob via DVE/ACT ops; per-expert counts & prob-sums are
    reduced over tokens with ones-matmuls and AllReduce'd across the 8 cores
    (the reference scales each token by the *global* mean gate prob of its
    expert). Routing slots dst[t] = cap*e + carry[tile, e] + rank[t, e] are
    built with two constant-matrix matmuls (strict-upper cumsum + tile-carry)
    and an indirect DMA scatter of token ids into a slot list.

  Phase B (FFN, float32r full-rate matmul, fp32 PSUM accumulate):
    for each expert (weights resident in SBUF), for each 256-token group:
      indirect-gather x rows -> PE-transpose -> xT tiles
      hT = relu(avg_e * (W1.T x + b1))  (scale+bias fused into the ACT copy)
      out = hT.T @ W2 + avg_e*b2        (bias via a K=1 ones-row matmul)
      indirect-scatter rows back to out[token, :] (padding slots are
      out-of-bounds and silently skipped).

The kernel returns (out[B, T, D], aux_loss) like the reference; aux_loss is
computed on host from the device-computed global counts.
"""

from contextlib import ExitStack

import numpy as np

import concourse.bass as bass
import concourse.mybir as mybir
from concourse.bass import IndirectOffsetOnAxis
from concourse.tile import TileContext

F32 = mybir.dt.float32
F32R = mybir.dt.float32r
I32 = mybir.dt.int32
AX = mybir.AxisListType
ALU = mybir.AluOpType
ACT = mybir.ActivationFunctionType

B, T, D, H, E = 8, 4096, 1024, 2048, 4
DT, HT = D // 128, H // 128
BIG = 1 << 20

N_CORES = 8
N_TOK = 4096          # tokens per core
CAP = 1536            # per-core per-expert slot capacity
GRP = 256             # token group size inside an expert

_MAX_WAITS = 1


class _SplitDrainTileContext(TileContext):
    """This toolchain's walrus build rejects instructions carrying more than
    one sync-wait ("Too many sync wait commands"); emit the final drain as a
    chain of single-wait drains."""

    def _drain_and_barrier(self, tick_clock, wait_clock):
        import bass_rust
        nc = self.nc
        drain_inst = nc.sync.drain()
        wait_clock.add_sem_waits(
            drain_inst.ins, bass_rust.ScopedClock({None: tick_clock.global_clock})
        )
        si = drain_inst.ins.sync_info
        if si is not None and si.on_wait and len(si.on_wait) > _MAX_WAITS:
            waits = list(si.on_wait)
            drain_inst.ins.sync_info = mybir.SyncInfo(
                on_wait=waits[:_MAX_WAITS], on_update=[])
            for i in range(_MAX_WAITS, len(waits), _MAX_WAITS):
                d2 = nc.sync.drain()
                d2.ins.sync_info = mybir.SyncInfo(
                    on_wait=waits[i:i + _MAX_WAITS], on_update=[])
        nc.all_engine_barrier()
        popped = nc._tile_sem_poison_stack.pop()
        assert popped is self._sem_poison
        nc.clear_and_free_semaphores(list(self.sems.allocated().values()))
        nc.all_engine_barrier()


def _make_nop(nc, engine):
    eng = nc.engines[engine]
    inst = eng.nop(nofuse=True)
    cur_insts = nc.cur_bb.bb.instructions
    assert cur_insts and cur_insts[-1].name == inst.ins.name
    new_list = list(cur_insts)
    new_list.pop()
    nc.cur_bb.bb.instructions = new_list
    return inst.ins


def _split_multi_waits(nc, max_waits=_MAX_WAITS):
    """Hoist overflow sem-waits of any instruction onto same-engine nops."""
    for _, bbb in list(nc.bb_map.items()):
        bb = bbb.bb
        out = []
        changed = False
        for inst in list(bb.instructions):
            si = inst.sync_info
            if si is not None and si.on_wait and len(si.on_wait) > max_waits:
                waits = list(si.on_wait)
                inst.sync_info = mybir.SyncInfo(
                    on_wait=waits[:max_waits], on_update=list(si.on_update or []))
                for i in range(max_waits, len(waits), max_waits):
                    nop = _make_nop(nc, inst.engine)
                    nop.sync_info = mybir.SyncInfo(
                        on_wait=waits[i:i + max_waits], on_update=[])
                    out.append(nop)
                changed = True
            out.append(inst)
        if changed:
            bb.instructions = out


def _bcast_last(ap, n):
    lst = [list(x) for x in ap.ap] + [[0, n]]
    return bass.AP(ap.tensor, ap.offset, lst)


def _build_host_consts(n_tok, cap):
    TT = n_tok // 128
    ident = np.eye(128, dtype=np.float32)
    SU = np.triu(np.ones((128, 128), np.float32), 1)
    CM = np.zeros((128, 128), np.float32)
    for Tp in range(TT):
        for ee in range(E):
            for Tc in range(TT):
                if Tp < Tc:
                    CM[Tp * 4 + ee, Tc * 4 + ee] = 1.0
    basec = np.zeros((128, 1), np.float32)
    for Tc in range(TT):
        for ee in range(E):
            basec[Tc * 4 + ee, 0] = cap * ee
    ones_col = np.ones((128, 1), np.float32)
    ones_row = np.ones((1, 128), np.float32)
    tokid = (np.arange(128)[:, None] + 128 * np.arange(TT)[None, :]).astype(np.int32)
    bigfill = np.full((128, (E * cap) // 128), BIG, np.int32)
    return dict(ident=ident, SU=SU, CM=CM, basec=basec,
                ones_col=ones_col, ones_row=ones_row, tokid=tokid,
                bigfill=bigfill)


def _build_moe_nc(n_tok=N_TOK, cap=CAP, grp=GRP, n_cores=N_CORES):
    TT = n_tok // 128
    NTT = n_tok // 512
    CAPB = cap // 128
    SLOTS = E * cap
    OFFC = SLOTS // 128
    GB = grp // 128
    NG = cap // grp

    nc = bass.Bass("TRN2", target_bir_lowering=False, debug=False,
                   num_devices=n_cores)

    def din(name, shape, dt):
        return nc.dram_tensor(name, shape, dt, kind="ExternalInput").ap()

    xT = din("xT", [D, n_tok], F32)
    x = din("x", [n_tok, D], F32R)
    Wg = din("Wg", [D, E], F32)
    bgT = din("bgT", [E, 1], F32)
    W1 = din("W1", [E, D, H], F32R)
    b1p = din("b1p", [E, 128, HT], F32)
    W2 = din("W2", [E, H, D], F32R)
    b2 = din("b2", [E, 1, D], F32)
    ident = din("ident", [128, 128], F32)
    identr = din("identr", [128, 128], F32R)
    SU = din("SU", [128, 128], F32)
    CM = din("CM", [128, 128], F32)
    basec = din("basec", [128, 1], F32)
    ones_col = din("ones_col", [128, 1], F32)
    ones_row = din("ones_row", [1, 128], F32)
    ones_rowr = din("ones_rowr", [1, 128], F32R)
    tokid = din("tokid", [128, TT], I32)
    bigfill = din("bigfill", [128, OFFC], I32)

    out = nc.dram_tensor("out", [n_tok, D], F32, kind="ExternalOutput").ap()
    ccout = nc.dram_tensor("ccout", [8, 1], F32, kind="ExternalOutput").ap()

    with _SplitDrainTileContext(nc) as tc, ExitStack() as ctx:
        ec = ctx.enter_context
        const_p = ec(tc.tile_pool(name="consts", bufs=1))
        dram_p = ec(tc.tile_pool(name="dram", bufs=1, space="DRAM"))
        route_p = ec(tc.tile_pool(name="route", bufs=1))

        def cload(name, shape, dt, src):
            t = const_p.tile(shape, dt, tag=name)
            nc.sync.dma_start(out=t[...] if len(shape) > 2 else t[:, :], in_=src)
            return t

        ident_sb = cload("ident", [128, 128], F32, ident)
        identr_sb = cload("identr", [128, 128], F32R, identr)
        SU_sb = cload("SU", [128, 128], F32, SU)
        CM_sb = cload("CM", [128, 128], F32, CM)
        basec_sb = cload("basec", [128, 1], F32, basec)
        onesc_sb = cload("onesc", [128, 1], F32, ones_col)
        onesr_sb = cload("onesr", [1, 128], F32, ones_row)
        onesrr_sb = cload("onesrr", [1, 128], F32R, ones_rowr)
        tokid_sb = cload("tokid", [128, TT], I32, tokid)
        bgT_sb = cload("bgT", [E, 1], F32, bgT)
        wg_sb = const_p.tile([128, DT, E], F32, tag="wg")
        for dt in range(DT):
            nc.sync.dma_start(out=wg_sb[:, dt, :], in_=Wg[dt * 128:(dt + 1) * 128, :])

        # ---------------- Phase A: gate ----------------
        with (
            tc.tile_pool(name="gx", bufs=16) as gx_p,
            tc.tile_pool(name="gps", bufs=2, space="PSUM") as gps_p,
            tc.tile_pool(name="glog", bufs=1) as glog_p,
            tc.tile_pool(name="gsm", bufs=1) as gsm_p,
        ):
            logT_sb = glog_p.tile([E, n_tok], F32, tag="logT")
            for st in range(NTT):
                xts = []
                for dt in range(DT):
                    xt = gx_p.tile([128, 512], F32, tag="gxt")
                    nc.sync.dma_start(
                        out=xt[:, :],
                        in_=xT[dt * 128:(dt + 1) * 128, st * 512:(st + 1) * 512])
                    xts.append(xt)
                ps_l = gps_p.tile([E, 512], F32, tag="psl")
                for dt in range(DT):
                    nc.tensor.matmul(ps_l[:, :], wg_sb[:, dt, :], xts[dt][:, :],
                                     start=(dt == 0), stop=(dt == DT - 1))
                nc.vector.tensor_scalar_add(
                    logT_sb[:, st * 512:(st + 1) * 512], ps_l[:, :], bgT_sb[:, :])

            L_sb = gsm_p.tile([128, TT, E], F32, tag="L")
            for Tc in range(TT):
                ps_t = gps_p.tile([128, E], F32, tag="pst")
                nc.tensor.matmul(ps_t[:, :], logT_sb[:, Tc * 128:(Tc + 1) * 128],
                                 ident_sb[0:E, 0:E], is_transpose=True,
                                 start=True, stop=True)
                nc.vector.tensor_copy(L_sb[:, Tc, :], ps_t[:, :])

            m_sb = gsm_p.tile([128, TT], F32, tag="m")
            nc.vector.tensor_reduce(m_sb[:, :], L_sb[:, :, :], AX.X, ALU.max)
            m_b = _bcast_last(m_sb[:, :], E)
            Lc_sb = gsm_p.tile([128, TT, E], F32, tag="Lc")
            nc.vector.tensor_tensor(Lc_sb[:, :, :], L_sb[:, :, :], m_b, ALU.subtract)
            E_sb = gsm_p.tile([128, TT, E], F32, tag="E")
            nc.scalar.activation(E_sb[:, :, :], Lc_sb[:, :, :], ACT.Exp)
            Z_sb = gsm_p.tile([128, TT], F32, tag="Z")
            nc.vector.tensor_reduce(Z_sb[:, :], E_sb[:, :, :], AX.X, ALU.add)
            p_sb = gsm_p.tile([128, TT], F32, tag="p")
            nc.vector.reciprocal(p_sb[:, :], Z_sb[:, :])

            eqr_sb = gsm_p.tile([128, TT, E], F32, tag="eqr")
            nc.vector.tensor_tensor(eqr_sb[:, :, :], L_sb[:, :, :], m_b, ALU.is_equal)
            eq_sb = gsm_p.tile([128, TT, E], F32, tag="eq")
            pre_sb = gsm_p.tile([128, TT], F32, tag="pre")
            z_sb = gsm_p.tile([128, TT], F32, tag="z")
            nc.vector.tensor_copy(eq_sb[:, :, 0], eqr_sb[:, :, 0])
            nc.vector.tensor_copy(pre_sb[:, :], eqr_sb[:, :, 0])
            for e in range(1, E):
                nc.vector.tensor_scalar(z_sb[:, :], pre_sb[:, :], 0.0, None,
                                        ALU.is_equal)
                nc.vector.tensor_mul(eq_sb[:, :, e], eqr_sb[:, :, e], z_sb[:, :])
                if e < E - 1:
                    nc.vector.tensor_add(pre_sb[:, :], pre_sb[:, :], eq_sb[:, :, e])

            eqp_sb = gsm_p.tile([128, E], F32, tag="eqp")
            nc.vector.tensor_reduce(eqp_sb[:, :],
                                    eq_sb[:, :, :].rearrange("p t e -> p e t"),
                                    AX.X, ALU.add)
            pw_sb = gsm_p.tile([128, TT, E], F32, tag="pw")
            nc.vector.tensor_tensor(pw_sb[:, :, :], eq_sb[:, :, :],
                                    _bcast_last(p_sb[:, :], E), ALU.mult)
            pwp_sb = gsm_p.tile([128, E], F32, tag="pwp")
            nc.vector.tensor_reduce(pwp_sb[:, :],
                                    pw_sb[:, :, :].rearrange("p t e -> p e t"),
                                    AX.X, ALU.add)

            ps_cnt = gps_p.tile([E, 1], F32, tag="gsmall")
            nc.tensor.matmul(ps_cnt[:, :], eqp_sb[:, :], onesc_sb[:, :],
                             start=True, stop=True)
            ps_sum = gps_p.tile([E, 1], F32, tag="gsmall")
            nc.tensor.matmul(ps_sum[:, :], pwp_sb[:, :], onesc_sb[:, :],
                             start=True, stop=True)
            cnt_l = route_p.tile([E, 1], F32, tag="cntl")
            nc.vector.tensor_copy(cnt_l[:, :], ps_cnt[:, :])
            sum_l = route_p.tile([E, 1], F32, tag="suml")
            nc.vector.tensor_copy(sum_l[:, :], ps_sum[:, :])

            cc_in = dram_p.tile([2 * E, 1], F32, tag="ccin")
            cc_sh = dram_p.tile([2 * E, 1], F32, tag="ccsh")
            nc.sync.dma_start(out=cc_in[0:E, :], in_=cnt_l[:, :])
            nc.sync.dma_start(out=cc_in[E:2 * E, :], in_=sum_l[:, :])
            nc.gpsimd.collective_compute(
                "AllReduce", ALU.add,
                replica_groups=[list(range(n_cores))],
                ins=[cc_in[:, :].opt()],
                outs=[cc_sh[:, :].opt()],
            )
            cnt_g = route_p.tile([E, 1], F32, tag="cntg")
            nc.sync.dma_start(out=cnt_g[:, :], in_=cc_sh[0:E, :])
            sum_g = route_p.tile([E, 1], F32, tag="sumg")
            nc.sync.dma_start(out=sum_g[:, :], in_=cc_sh[E:2 * E, :])
            nc.sync.dma_start(out=ccout[:, :], in_=cc_sh[:, :])

            cmax_sb = route_p.tile([E, 1], F32, tag="cmax")
            nc.vector.tensor_scalar_max(cmax_sb[:, :], cnt_g[:, :], 1.0)
            rec_sb = route_p.tile([E, 1], F32, tag="rec")
            nc.vector.reciprocal(rec_sb[:, :], cmax_sb[:, :])
            avg_sb = route_p.tile([E, 1], F32, tag="avg")
            nc.vector.tensor_mul(avg_sb[:, :], sum_g[:, :], rec_sb[:, :])
            avgd = dram_p.tile([E, 1], F32, tag="avgd")
            nc.sync.dma_start(out=avgd[:, :], in_=avg_sb[:, :])
            avgT_sb = route_p.tile([1, E], F32, tag="avgT")
            nc.sync.dma_start(out=avgT_sb[:, :],
                              in_=avgd[:, :].rearrange("a b -> b a"))
            ps_ab = gps_p.tile([128, E], F32, tag="gsmall")
            nc.tensor.matmul(ps_ab[:, :], onesr_sb[:, :], avgT_sb[:, :],
                             start=True, stop=True)
            avg_bc = route_p.tile([128, E], F32, tag="avgbc")
            nc.vector.tensor_copy(avg_bc[:, :], ps_ab[:, :])

            eq_flat = eq_sb[:, :, :].rearrange("p t e -> p (t e)")
            ps_rk = gps_p.tile([128, TT * E], F32, tag="gsmall")
            nc.tensor.matmul(ps_rk[:, :], SU_sb[:, :], eq_flat, start=True, stop=True)
            rank_sb = gsm_p.tile([128, TT, E], F32, tag="rank")
            nc.vector.tensor_copy(rank_sb[:, :, :].rearrange("p t e -> p (t e)"),
                                  ps_rk[:, :])

            ps_ts = gps_p.tile([TT * E, 1], F32, tag="gsmall")
            nc.tensor.matmul(ps_ts[:, :], eq_flat, onesc_sb[:, :],
                             start=True, stop=True)
            ts_sb = route_p.tile([128, 1], F32, tag="ts")
            nc.vector.memset(ts_sb[:, :], 0.0)
            nc.vector.tensor_copy(ts_sb[0:TT * E, :], ps_ts[:, :])
            ps_cy = gps_p.tile([128, 1], F32, tag="gsmall")
            nc.tensor.matmul(ps_cy[:, :], CM_sb[:, :], ts_sb[:, :],
                             start=True, stop=True)
            bc_sb = route_p.tile([128, 1], F32, tag="bc")
            nc.vector.tensor_tensor(bc_sb[:, :], ps_cy[:, :], basec_sb[:, :], ALU.add)
            bcd = dram_p.tile([128, 1], F32, tag="bcd")
            nc.sync.dma_start(out=bcd[:, :], in_=bc_sb[:, :])
            bcT_sb = route_p.tile([1, 128], F32, tag="bcT")
            nc.sync.dma_start(out=bcT_sb[:, :],
                              in_=bcd[:, :].rearrange("a b -> b a"))
            ps_bb = gps_p.tile([128, 128], F32, tag="gsmall")
            nc.tensor.matmul(ps_bb[:, :], onesr_sb[:, :], bcT_sb[:, :],
                             start=True, stop=True)
            bcb_sb = gsm_p.tile([128, TT, E], F32, tag="bcb")
            nc.vector.tensor_copy(bcb_sb[:, :, :].rearrange("p t e -> p (t e)"),
                                  ps_bb[:, 0:TT * E])

            t1_sb = gsm_p.tile([128, TT, E], F32, tag="t1")
            nc.vector.tensor_add(t1_sb[:, :, :], rank_sb[:, :, :], bcb_sb[:, :, :])
            nc.vector.tensor_mul(t1_sb[:, :, :], t1_sb[:, :, :], eq_sb[:, :, :])
            dstf_sb = gsm_p.tile([128, TT], F32, tag="dstf")
            nc.vector.tensor_reduce(dstf_sb[:, :], t1_sb[:, :, :], AX.X, ALU.add)
            dsti_sb = route_p.tile([128, TT], I32, tag="dsti")
            nc.vector.tensor_copy(dsti_sb[:, :], dstf_sb[:, :])

            listd = dram_p.tile([SLOTS], I32, tag="listd")
            big_sb = route_p.tile([128, OFFC], I32, tag="bigsb")
            nc.sync.dma_start(out=big_sb[:, :], in_=bigfill)
            nc.sync.dma_start(out=listd[:].rearrange("(p c) -> p c", p=128),
                              in_=big_sb[:, :])
            # NOTE: indirect DMAs only behave on HW with 2-D APs and a
            # single-column [128, 1] offset vector -- issue one per column.
            for Tc in range(TT):
                nc.gpsimd.indirect_dma_start(
                    out=listd[:].rearrange("(s one) -> s one", one=1),
                    out_offset=IndirectOffsetOnAxis(ap=dsti_sb[:, Tc:Tc + 1],
                                                    axis=0),
                    in_=tokid_sb[:, Tc:Tc + 1],
                    in_offset=None,
                    bounds_check=SLOTS - 1,
                    oob_is_err=False,
                )
            offs_sb = route_p.tile([128, OFFC], I32, tag="offs")
            nc.sync.dma_start(out=offs_sb[:, :],
                              in_=listd[:].rearrange("(c p) -> p c", p=128))

        # ---------------- Phase B: FFN ----------------
        tok_bnd = nc.gpsimd.to_reg(n_tok - 1)
        with (
            tc.tile_pool(name="w1", bufs=DT) as w1_p,
            tc.tile_pool(name="w2", bufs=HT) as w2_p,
            tc.tile_pool(name="bias", bufs=2) as bias_p,
            tc.tile_pool(name="xg", bufs=2) as xg_p,
            tc.tile_pool(name="xgT", bufs=1) as xgT_p,
            tc.tile_pool(name="hT", bufs=1) as hT_p,
            tc.tile_pool(name="og", bufs=1) as og_p,
            tc.tile_pool(name="psT", bufs=2, space="PSUM") as psT_p,
            tc.tile_pool(name="psH", bufs=2, space="PSUM") as psH_p,
            tc.tile_pool(name="psO", bufs=2, space="PSUM") as psO_p,
        ):
            for e in range(E):
                b1p_sb = bias_p.tile([128, HT], F32, tag="b1p")
                nc.sync.dma_start(out=b1p_sb[:, :], in_=b1p[e, :, :])
                b1s_sb = bias_p.tile([128, HT], F32, tag="b1s")
                nc.vector.tensor_scalar_mul(b1s_sb[:, :], b1p_sb[:, :],
                                            avg_bc[:, e:e + 1])
                b2_sb = bias_p.tile([1, D], F32, tag="b2")
                nc.sync.dma_start(out=b2_sb[:, :], in_=b2[e, :, :])
                b2s_sb = bias_p.tile([1, D], F32R, tag="b2s")
                nc.vector.tensor_scalar_mul(b2s_sb[:, :], b2_sb[:, :],
                                            avg_bc[0:1, e:e + 1])

                w1t = []
                for dt in range(DT):
                    t = w1_p.tile([128, H], F32R, tag="w1")
                    nc.sync.dma_start(out=t[:, :],
                                      in_=W1[e, dt * 128:(dt + 1) * 128, :])
                    w1t.append(t)
                w2t = []
                for ht in range(HT):
                    t = w2_p.tile([128, D], F32R, tag="w2")
                    nc.sync.dma_start(out=t[:, :],
                                      in_=W2[e, ht * 128:(ht + 1) * 128, :])
                    w2t.append(t)

                NG_ = NG
                GB_ = GB
                for g in range(NG_):
                    c0 = CAPB * e + GB_ * g
                    offsl = offs_sb[:, c0:c0 + GB_]
                    xg_t = xg_p.tile([128, GB_, D], F32R, tag="xg")
                    for b in range(GB_):
                        nc.gpsimd.indirect_dma_start(
                            out=xg_t[:, b, :],
                            out_offset=None,
                            in_=x,
                            in_offset=IndirectOffsetOnAxis(
                                ap=offsl[:, b:b + 1], axis=0),
                            bounds_check=tok_bnd,
                            oob_is_err=False,
                        )
                    xgT_t = xgT_p.tile([128, DT, grp], F32R, tag="xgT")
                    for dt in range(DT):
                        ps_t = psT_p.tile([128, grp], F32R, tag="psT")
                        for b in range(GB_):
                            nc.tensor.matmul(
                                ps_t[:, b * 128:(b + 1) * 128],
                                xg_t[:, b, dt * 128:(dt + 1) * 128],
                                identr_sb[:, :], is_transpose=True,
                                start=(b == 0), stop=(b == GB_ - 1))
                        nc.vector.tensor_copy(xgT_t[:, dt, :], ps_t[:, :])
                    hT_t = hT_p.tile([128, HT, grp], F32R, tag="hT")
                    for ht in range(HT):
                        ps_h = psH_p.tile([128, grp], F32, tag="psH")
                        for dt in range(DT):
                            nc.tensor.matmul(
                                ps_h[:, :],
                                w1t[dt][:, ht * 128:(ht + 1) * 128],
                                xgT_t[:, dt, :],
                                start=(dt == 0), stop=(dt == DT - 1))
                        nc.scalar.activation(hT_t[:, ht, :], ps_h[:, :], ACT.Relu,
                                             bias=b1s_sb[:, ht:ht + 1],
                                             scale=avg_bc[:, e:e + 1])
                    og_t = og_p.tile([128, GB_, D], F32, tag="og")
                    for b in range(GB_):
                        for dh in range(2):
                            ps_o = psO_p.tile([128, 512], F32, tag="psO")
                            for ht in range(HT):
                                nc.tensor.matmul(
                                    ps_o[:, :],
                                    hT_t[:, ht, b * 128:(b + 1) * 128],
                                    w2t[ht][:, dh * 512:(dh + 1) * 512],
                                    start=(ht == 0), stop=False)
                            nc.tensor.matmul(
                                ps_o[:, :], onesrr_sb[:, :],
                                b2s_sb[:, dh * 512:(dh + 1) * 512],
                                start=False, stop=True)
                            nc.scalar.activation(
                                og_t[:, b, dh * 512:(dh + 1) * 512],
                                ps_o[:, :], ACT.Copy)
                    for b in range(GB_):
                        nc.gpsimd.indirect_dma_start(
                            out=out,
                            out_offset=IndirectOffsetOnAxis(
                                ap=offsl[:, b:b + 1], axis=0),
                            in_=og_t[:, b, :],
                            in_offset=None,
                            bounds_check=tok_bnd,
                            oob_is_err=False,
                        )

    _split_multi_waits(nc)
    return nc


def _host_inputs_for_core(xc, Wg, bg, W1, b1, W2, b2, n_tok, cap):
    consts = _build_host_consts(n_tok, cap)
    b1p = np.ascontiguousarray(
        b1.reshape(E, HT, 128).transpose(0, 2, 1)).astype(np.float32)
    return {
        "xT": np.ascontiguousarray(xc.T),
        "x": np.ascontiguousarray(xc),
        "Wg": Wg, "bgT": bg.reshape(E, 1),
        "W1": W1, "b1p": b1p, "W2": W2, "b2": b2.reshape(E, 1, D),
        "ident": consts["ident"], "identr": consts["ident"],
        "SU": consts["SU"], "CM": consts["CM"], "basec": consts["basec"],
        "ones_col": consts["ones_col"], "ones_row": consts["ones_row"],
        "ones_rowr": consts["ones_row"],
        "tokid": consts["tokid"], "bigfill": consts["bigfill"],
    }


_CACHED = {}


def _get_nc():
    if "nc" not in _CACHED:
        _CACHED["nc"] = _build_moe_nc()
    return _CACHED["nc"]


def kernel(x, Wg, bg, W1, b1, W2, b2):
    from concourse.bass_utils import run_bass_kernel_spmd

    x = np.asarray(x, dtype=np.float32)
    Wg = np.asarray(Wg, dtype=np.float32)
    bg = np.asarray(bg, dtype=np.float32)
    W1 = np.asarray(W1, dtype=np.float32)
    b1 = np.asarray(b1, dtype=np.float32)
    W2 = np.asarray(W2, dtype=np.float32)
    b2 = np.asarray(b2, dtype=np.float32)

    Bs, Ts, Ds = x.shape
    xf = x.reshape(-1, Ds)
    N = xf.shape[0]
    assert N == N_CORES * N_TOK and Ds == D

    # Host-side sanity check of the static per-expert capacity (the gate is
    # recomputed on device; this only validates the routing fits).
    logits = xf @ Wg + bg
    idx = logits.argmax(-1)
    for c in range(N_CORES):
        pc = np.bincount(idx[c * N_TOK:(c + 1) * N_TOK], minlength=E)
        if pc.max() > CAP:
            raise RuntimeError(f"per-core expert load {pc} exceeds CAP={CAP}")

    nc = _get_nc()
    in_maps = [
        _host_inputs_for_core(xf[c * N_TOK:(c + 1) * N_TOK], Wg, bg, W1, b1,
                              W2, b2, N_TOK, CAP)
        for c in range(N_CORES)
    ]
    res = run_bass_kernel_spmd(nc, in_maps, core_ids=list(range(N_CORES)))

    out = np.concatenate([res.results[c]["out"] for c in range(N_CORES)], 0)
    counts = res.results[0]["ccout"].ravel()[:E].astype(np.float32)
    aux = np.float32(np.sum((counts / np.float32(N)) ** 2) * E)
    return out.reshape(Bs, Ts, Ds), aux
